# revision 28
# baseline (speedup 1.0000x reference)
"""Trainium2 Bass kernel for the DfOp deep-filtering module.

out[b, t, f<96]  = sum_{k=0..4} coefs[b, k, t, f] (*) spec[b, t-4+k, f]   (complex mult)
out[b, t, f>=96] = spec[b, t, f]                                          (passthrough)

Sharding: data-parallel over batch B=8 -> one batch element per NeuronCore.

The hi-band (385 of 481 bins) is a pure passthrough, so it never touches the
device: the host copies it straight into the output during unshard.  The
device only sees the lo band, in fp16 (the 2e-2 gate leaves ~40x margin),
shrinking per-core HBM traffic from ~47.6 MB to ~10.6 MB.

Host packs, per core, partition-major fp16 buffers (partition p owns the 32
timesteps [32p, 32p+32)):
  sp[p]  = [ s_re rows 32p-4..32p+32 | s_im rows ... ]   (36x96 each, zero-pad t<0)
  cf[p]  = per tap k: [ c_re k | c_im k ]                (32x96 each)
so every DMA is a [128, N] contiguous load and the causal 5-tap window is a
pure free-dim offset into the sp tile -- no on-chip halo exchange at all.

Compute per 16-step time half, per tap k, two fused DVE tensor_muls with
2-run access patterns (fp16 unit-stride inner => 2x_1P mode, FD=3072 each to
amortize the ~150-cycle DVE fixed cost):
  A = [sr|si] * [cr|ci] = [rr|ii]
  B = [si|sr] * [cr|ci] = [ir|ri]      (B's in0 reverses the planes via a
                                        negative outer AP stride -- no copy)
The first taps are issued as 4 plain half-size tensor_muls instead, so DVE
starts as soon as the first half-plane DMA lands.  PE accumulates product
halves into fp32 PSUM via identity matmuls (negated identity gives rr-ii):
  ps_re = sum_k I@rr_k + (-I)@ii_k,   ps_im = sum_k I@ir_k + I@ri_k
Matmuls are emitted stream-major so consecutive MMs hit different PSUM banks
(hides the ~173ns PSUM access latency).  ACT drains PSUM -> fp16 out tile;
per-half-per-plane stores overlap the next half's compute.
"""

import sys

import numpy as np

try:
    import concourse.bacc  # noqa: F401  (resolves via the environment's path)
except ImportError:  # pragma: no cover - fallback for bare environments
    for _p in ("/opt/trn_rl_repo", "/root/.axon_site/_ro/trn_rl_repo"):
        if _p not in sys.path:
            sys.path.append(_p)

import concourse.bacc as bacc
import concourse.mybir as mybir
from concourse.tile import TileContext
from concourse.bass_utils import run_bass_kernel_spmd

B = 8          # batch / cores
T = 4096       # time steps
F = 481        # total freq bins
NF = 96        # deep-filtered freq bins
FS = 5         # frame size (causal taps)
HL = FS - 1    # halo slots (4)
P = 128        # partitions
TB = T // P    # timesteps per partition block   (32)
NH = 2         # time halves per block
TI = TB // NH  # timesteps per half              (16)
SW = TB + HL   # spec rows held per partition    (36)
SPL = SW * NF  # spec plane elems per partition  (3456)
CPL = TB * NF  # coef plane elems per partition  (3072)
CW = TI * NF   # half cols                       (1536)
NSPLIT = 2     # leading (k, h=0) taps issued as plain half-size TTs

_nc_cache = None


def _body(nc, tc, sp_d, cf_d, id_d, out_d):
    f16 = mybir.dt.float16
    f32 = mybir.dt.float32

    with (
        tc.tile_pool(name="const", bufs=1) as cpool,
        tc.tile_pool(name="spec", bufs=1) as spool,
        tc.tile_pool(name="coef", bufs=2 * FS) as kpool,
        tc.tile_pool(name="out", bufs=1) as opool,
        tc.tile_pool(name="prod", bufs=10) as ppool,
        tc.tile_pool(name="psum", bufs=2, space="PSUM") as pspool,
    ):
        id_sb = cpool.tile([P, 2 * P], f16)
        nc.sync.dma_start(out=id_sb[:], in_=id_d)
        ident = id_sb[:, 0:P]
        negid = id_sb[:, P:2 * P]

        ot_sb = opool.tile([P, 2 * TB * NF], f16)
        sp_sb = spool.tile([P, 2 * SPL], f16)
        spv = sp_sb[:].rearrange("p (c x) -> p c x", c=2)

        cf_sb = [[None] * NH for _ in range(FS)]
        cdv = cf_d.rearrange("p (k c x) -> p k c x", k=FS, c=2)

        def load_coef(k, h, split):
            t_ = kpool.tile([P, 2 * CW], f16, tag="coef")
            if split == 2:  # quarter-plane DMAs for the earliest tap
                for a in range(0, 2 * CW, 768):
                    nc.sync.dma_start(
                        out=t_[:, a:a + 768],
                        in_=cdv[:, k, a // CW, h * CW + a % CW:
                                h * CW + a % CW + 768])
            elif split == 1:  # cr half then ci half as separate DMAs
                nc.sync.dma_start(out=t_[:, 0:CW],
                                  in_=cdv[:, k, 0, h * CW:(h + 1) * CW])
                nc.sync.dma_start(out=t_[:, CW:2 * CW],
                                  in_=cdv[:, k, 1, h * CW:(h + 1) * CW])
            else:
                nc.sync.dma_start(
                    out=t_[:].rearrange("p (c x) -> p c x", c=2),
                    in_=cdv[:, k, :, h * CW:(h + 1) * CW],
                )
            cf_sb[k][h] = t_

        # load order == consumption order (sync HWDGE ring is FIFO):
        # h0 rows of sr first, then the early taps' planes interleaved so DVE
        # starts as soon as possible; h1 spec rows arrive mid-stream.
        HR = (TI + HL) * NF                                   # rows 0..20
        nc.sync.dma_start(out=sp_sb[:, 0:HR], in_=sp_d[:, 0:HR])
        load_coef(0, 0, 1)
        nc.sync.dma_start(out=sp_sb[:, SPL:SPL + HR],
                          in_=sp_d[:, SPL:SPL + HR])
        load_coef(1, 0, 1)
        load_coef(2, 0, False)
        load_coef(3, 0, False)
        nc.sync.dma_start(out=sp_sb[:, HR:SPL], in_=sp_d[:, HR:SPL])
        nc.sync.dma_start(out=sp_sb[:, SPL + HR:2 * SPL],
                          in_=sp_d[:, SPL + HR:2 * SPL])
        load_coef(4, 0, False)
        for k in range(FS):
            load_coef(k, 1, False)

        otv = ot_sb[:].rearrange("p (c x) -> p c x", c=2)
        odv = out_d.rearrange("p (c x) -> p c x", c=2)

        def prod_a(h, k):
            w = (TI * h + k) * NF
            cf = cf_sb[k][h][:]
            pa = ppool.tile([P, 2 * CW], f16, tag="prod")       # [rr | ii]
            if h == 0 and k < NSPLIT:
                sr = sp_sb[:, w:w + CW]
                si = sp_sb[:, SPL + w:SPL + w + CW]
                nc.vector.tensor_mul(out=pa[:, 0:CW], in0=sr,
                                     in1=cf[:, 0:CW])           # rr
                nc.vector.tensor_mul(out=pa[:, CW:2 * CW], in0=si,
                                     in1=cf[:, CW:2 * CW])      # ii
            else:
                nc.vector.tensor_mul(
                    out=pa[:].rearrange("p (c x) -> p c x", c=2),
                    in0=spv[:, 0:2, w:w + CW],
                    in1=cf.rearrange("p (c x) -> p c x", c=2),
                )
            return pa

        def prod_b(h, k):
            w = (TI * h + k) * NF
            cf = cf_sb[k][h][:]
            pb = ppool.tile([P, 2 * CW], f16, tag="prod")       # [ir | ri]
            if h == 0 and k < NSPLIT:
                sr = sp_sb[:, w:w + CW]
                si = sp_sb[:, SPL + w:SPL + w + CW]
                nc.vector.tensor_mul(out=pb[:, 0:CW], in0=si,
                                     in1=cf[:, 0:CW])           # ir
                nc.vector.tensor_mul(out=pb[:, CW:2 * CW], in0=sr,
                                     in1=cf[:, CW:2 * CW])      # ri
            else:
                nc.vector.tensor_mul(
                    out=pb[:].rearrange("p (c x) -> p c x", c=2),
                    in0=spv[:, ::-1, w:w + CW],
                    in1=cf.rearrange("p (c x) -> p c x", c=2),
                )
            return pb

        def mm_pair(ps, lo_w, hi_w, p, k):
            # stream-major: consecutive MMs hit different PSUM banks
            for a in range(0, CW, 512):
                nc.tensor.matmul(ps[:, a:a + 512], lo_w, p[:, a:a + 512],
                                 start=(k == 0), stop=False)
            for a in range(0, CW, 512):
                nc.tensor.matmul(ps[:, a:a + 512], hi_w,
                                 p[:, CW + a:CW + a + 512],
                                 start=False, stop=(k == FS - 1))

        def drain(h, c, ps, vec_mid):
            for a in range(0, CW, 512):
                dst = otv[:, c, h * CW + a:h * CW + a + 512]
                if vec_mid and a == 512:
                    # DVE idle once the final products are out; offload the
                    # middle slice so ACT doesn't delay the final slice
                    nc.vector.tensor_copy(out=dst, in_=ps[:, a:a + 512])
                else:
                    nc.scalar.copy(out=dst, in_=ps[:, a:a + 512])
                nc.sync.dma_start(
                    out=odv[:, c, h * CW + a:h * CW + a + 512], in_=dst)

        # first half: A/B interleaved per tap (matches the DMA feed);
        # ps_im closes (ri, stop) before ps_re (ii, stop).
        ps_re = pspool.tile([P, CW], f32, tag="ps")
        ps_im = pspool.tile([P, CW], f32, tag="ps")
        for k in range(FS):
            if k < NSPLIT:  # plane-at-a-time in DMA arrival order
                w = k * NF
                cf = cf_sb[k][0][:]
                sr = sp_sb[:, w:w + CW]
                si = sp_sb[:, SPL + w:SPL + w + CW]
                pa = ppool.tile([P, 2 * CW], f16, tag="prod")
                pb = ppool.tile([P, 2 * CW], f16, tag="prod")
                nc.vector.tensor_mul(out=pa[:, 0:CW], in0=sr,
                                     in1=cf[:, 0:CW])           # rr
                nc.vector.tensor_mul(out=pb[:, CW:2 * CW], in0=sr,
                                     in1=cf[:, CW:2 * CW])      # ri
                nc.vector.tensor_mul(out=pa[:, CW:2 * CW], in0=si,
                                     in1=cf[:, CW:2 * CW])      # ii
                nc.vector.tensor_mul(out=pb[:, 0:CW], in0=si,
                                     in1=cf[:, 0:CW])           # ir
            else:
                pa = prod_a(0, k)
                pb = prod_b(0, k)
            mm_pair(ps_re, ident, negid, pa, k)
            mm_pair(ps_im, ident, ident, pb, k)
        drain(0, 1, ps_im, False)
        drain(0, 0, ps_re, False)

        # last half is phase-split: all A products first so ps_re closes and
        # drains while the B phase still computes -- only the im chain
        # remains after the final DVE instruction.
        ps_re = pspool.tile([P, CW], f32, tag="ps")
        ps_im = pspool.tile([P, CW], f32, tag="ps")
        for k in range(FS):
            mm_pair(ps_re, ident, negid, prod_a(1, k), k)
        drain(1, 0, ps_re, False)
        for k in range(FS):
            mm_pair(ps_im, ident, ident, prod_b(1, k), k)
        drain(1, 1, ps_im, True)


def _build_nc():
    nc = bacc.Bacc("TRN2", target_bir_lowering=False, debug=False, num_devices=B)
    f16 = mybir.dt.float16
    sp_d = nc.dram_tensor("sp", [P, 2 * SPL], f16, kind="ExternalInput").ap()
    cf_d = nc.dram_tensor("cf", [P, 2 * FS * CPL], f16, kind="ExternalInput").ap()
    id_d = nc.dram_tensor("id2", [P, 2 * P], f16, kind="ExternalInput").ap()
    out_d = nc.dram_tensor("out", [P, 2 * TB * NF], f16, kind="ExternalOutput").ap()
    with TileContext(nc) as tc:
        _body(nc, tc, sp_d, cf_d, id_d, out_d)
    nc.compile()
    return nc


def _in_maps(spec, coefs):
    spec = np.asarray(spec)
    coefs = np.asarray(coefs)
    id2 = np.concatenate(
        [np.eye(P, dtype=np.float16), -np.eye(P, dtype=np.float16)], axis=1
    )
    id2 = np.ascontiguousarray(id2)
    widx = np.arange(P)[:, None] * TB + np.arange(SW)[None, :]  # [128, 36]
    maps = []
    for b in range(B):
        lo = spec[b, 0, :, :NF, :].astype(np.float16)           # [T, 96, 2]
        pad = np.zeros((HL, NF, 2), dtype=np.float16)
        lop = np.concatenate([pad, lo], axis=0)                 # [T+4, 96, 2]
        win = lop[widx].transpose(0, 3, 1, 2)                   # [128, 2, 36, 96]
        sp = np.ascontiguousarray(win.reshape(P, 2 * SPL))
        cf = np.ascontiguousarray(
            coefs[b].astype(np.float16)                         # [5, T, 96, 2]
            .reshape(FS, P, TB, NF, 2)
            .transpose(1, 0, 4, 2, 3)                           # [128, 5, 2, 32, 96]
            .reshape(P, 2 * FS * CPL)
        )
        maps.append({"sp": sp, "cf": cf, "id2": id2})
    return maps


def kernel(spec, coefs):
    global _nc_cache
    if _nc_cache is None:
        _nc_cache = _build_nc()
    res = run_bass_kernel_spmd(_nc_cache, _in_maps(spec, coefs),
                               core_ids=list(range(B)))
    spec = np.asarray(spec, dtype=np.float32)
    out = np.empty((B, 1, T, F, 2), dtype=np.float32)
    out[:, :, :, NF:, :] = spec[:, :, :, NF:, :]
    for b in range(B):
        ot = res.results[b]["out"].reshape(P, 2, TB, NF)
        out[b, 0, :, :NF, 0] = ot[:, 0].reshape(T, NF).astype(np.float32)
        out[b, 0, :, :NF, 1] = ot[:, 1].reshape(T, NF).astype(np.float32)
    return out
